# revision 9
# baseline (speedup 1.0000x reference)
"""MultiProbeAttentionPooler Trainium2 kernel.

  logits[b,t,p] = x[b,t,:] @ W[p,:] + b[p]
  att = softmax(logits, axis=t)          -> returned as [B, P, T]
  pooled[b,p,d] = sum_t att[b,t,p] x[b,t,d]

Sharding: T is split across the 8 cores (512 tokens each); every core
handles all 16 batches.  Softmax normalization is deferred to the host:
each core returns unnormalized exp(logits) tiles, per-probe partial sums
Z, and unnormalized partial pooled sums; the host reduces/divides.

The TensorE contracts over the partition dim, so the logits matmul needs
x with D on partitions while the pooling matmul needs T on partitions.
The kernel is HBM-bound, so input bytes are minimized: x is fed in fp16
(PE: 1 cycle/row vs 4 for fp32), once naturally and once transposed -
but only 3/4 of the transposed copy comes from HBM; the last quarter is
rebuilt on-chip with PE transpose ops from the natural tiles, trading
idle PE cycles for DMA bytes.  Logits accumulate in fp32 PSUM; exp runs
in fp32 on ScalarE.  fp16's 10-bit mantissa keeps overall error ~5e-4.

Per core, per (batch, T-shard) unit of work:
  - DMA x_nat [512, 1024] (1 MB) and xT [1024, 384] (0.75 MB), fp16,
    one HWDGE ring each (SP / ACT).
  - PE transposes rebuild xT tokens 384:512 from x_nat (fp16 PSUM),
    VectorE copies them into the xT tile.
  - mm1: logitsT[p, t] += WT[dchunk].T @ xT[dchunk]  (8 chunks, fp32 PSUM)
  - ScalarE Exp with per-partition bias: E (fp16) plus its row-sums Z
    (accum_out, fp32) in one pass; E tiles stream out as the att output.
  - E chunks transposed (tiny fp16 PE ops) to [t, p] stationary layout.
  - mm2: pooled_raw[p, d] += eT[tblk].T @ x_nat[tblk]  (fp32 PSUM).
"""

import sys

import numpy as np

for _p in ("/opt/trn_rl_repo", "/root/.axon_site/_ro/trn_rl_repo"):
    if _p not in sys.path:
        sys.path.append(_p)

import concourse.bacc as bacc
import concourse.tile as tile
from concourse import mybir
from concourse.bass_utils import run_bass_kernel_spmd

dt = mybir.dt
AF = mybir.ActivationFunctionType

B, T, D, P = 16, 4096, 1024, 16
N_CORES = 8
TSH = T // N_CORES          # tokens per core (512)
TBLK = TSH // 128           # 128-token blocks per unit (4)
DSUB = D // 128             # 128-wide D chunks (8)
TFEED = 3 * TSH // 4        # transposed tokens fed from host (384)

_CACHE = {}


def _build():
    nc = bacc.Bacc("TRN2", target_bir_lowering=False, debug=False,
                   num_devices=N_CORES)

    # pre-tiled on host: each SBUF partition reads one contiguous run
    xn = nc.dram_tensor("xn", [B, 128, TBLK, D], dt.float16, kind="ExternalInput").ap()
    xt = nc.dram_tensor("xt", [B, 128, DSUB, TFEED], dt.float16, kind="ExternalInput").ap()
    wt = nc.dram_tensor("wt", [128, DSUB, P], dt.float16, kind="ExternalInput").ap()
    bb = nc.dram_tensor("bb", [P, 1], dt.float32, kind="ExternalInput").ap()
    idn = nc.dram_tensor("idn", [128, 128], dt.float16, kind="ExternalInput").ap()

    att_raw = nc.dram_tensor("att_raw", [B, P, TSH], dt.float16, kind="ExternalOutput").ap()
    zc = nc.dram_tensor("zc", [P, B], dt.float32, kind="ExternalOutput").ap()
    pooled_c = nc.dram_tensor("pooled_c", [B, P, D], dt.float32, kind="ExternalOutput").ap()

    with tile.TileContext(nc) as tc:
        with (
            tc.tile_pool(name="consts", bufs=1) as consts,
            tc.tile_pool(name="xn_p", bufs=3) as xn_p,
            tc.tile_pool(name="xt_p", bufs=3) as xt_p,
            tc.tile_pool(name="e_p", bufs=3) as e_p,
            tc.tile_pool(name="et_p", bufs=2) as et_p,
            tc.tile_pool(name="pl_p", bufs=2) as pl_p,
            tc.tile_pool(name="ps_tr", bufs=2, space="PSUM") as ps_tr,
            tc.tile_pool(name="ps_lg", bufs=2, space="PSUM") as ps_lg,
            tc.tile_pool(name="ps_et", bufs=2, space="PSUM") as ps_et,
            tc.tile_pool(name="ps_pl", bufs=1, space="PSUM") as ps_pl,
        ):
            wt_sb = consts.tile([128, DSUB, P], dt.float16)
            b_sb = consts.tile([P, 1], dt.float32)
            id_sb = consts.tile([128, 128], dt.float16)
            z_sb = consts.tile([P, B], dt.float32)
            nc.sync.dma_start(out=wt_sb, in_=wt)
            nc.sync.dma_start(out=b_sb, in_=bb)
            nc.sync.dma_start(out=id_sb, in_=idn)

            for b in range(B):
                # ---- load x slice both ways; separate HWDGE rings ----
                xn_tile = xn_p.tile([128, TBLK, D], dt.float16)
                nc.sync.dma_start(out=xn_tile, in_=xn[b])
                xt_tile = xt_p.tile([128, DSUB, TSH], dt.float16)
                nc.scalar.dma_start(out=xt_tile[:, :, :TFEED], in_=xt[b])

                # ---- rebuild xT tokens TFEED:TSH on-chip (t-block 3) ----
                for dg in range(2):          # groups of 4 D-chunks
                    pt = ps_tr.tile([128, 4, 128], dt.float16)
                    for i in range(4):
                        ds = dg * 4 + i
                        nc.tensor.transpose(
                            pt[:, i],
                            xn_tile[:, TBLK - 1, ds * 128:(ds + 1) * 128],
                            id_sb,
                        )
                    nc.vector.tensor_copy(
                        xt_tile[:, dg * 4:(dg + 1) * 4, TFEED:TSH], pt)

                # ---- mm1: logitsT [P, TSH] ----
                ps_l = ps_lg.tile([P, TSH], dt.float32)
                for ds in range(DSUB):
                    nc.tensor.matmul(ps_l, wt_sb[:, ds], xt_tile[:, ds],
                                     start=(ds == 0), stop=(ds == DSUB - 1))

                # ---- exp(logits + b) -> E (fp16), Z partial via accum_out ----
                e_sb = e_p.tile([P, TSH], dt.float16)
                nc.scalar.activation(e_sb, ps_l, AF.Exp, bias=b_sb, scale=1.0,
                                     accum_out=z_sb[:, b:b + 1])
                nc.scalar.dma_start(out=att_raw[b], in_=e_sb)

                # ---- E chunks back to [t, p] (fp16) for mm2 stationary ----
                et_sb = et_p.tile([128, TBLK, P], dt.float16)
                for tb in range(TBLK):
                    pe = ps_et.tile([128, P], dt.float16)
                    nc.tensor.transpose(pe, e_sb[:, tb * 128:(tb + 1) * 128],
                                        id_sb[:P, :P])
                    nc.vector.tensor_copy(et_sb[:, tb], pe)

                # ---- mm2: pooled_raw [P, D] ----
                ps_p = ps_pl.tile([P, D], dt.float32)
                for tb in range(TBLK):
                    for dh in range(2):
                        nc.tensor.matmul(
                            ps_p[:, dh * 512:(dh + 1) * 512],
                            et_sb[:, tb],
                            xn_tile[:, tb, dh * 512:(dh + 1) * 512],
                            start=(tb == 0), stop=(tb == TBLK - 1),
                        )

                pl_sb = pl_p.tile([P, D], dt.float32)
                if b % 2 == 0:
                    nc.vector.tensor_copy(pl_sb, ps_p)
                else:
                    nc.scalar.activation(pl_sb, ps_p, AF.Copy)
                nc.sync.dma_start(out=pooled_c[b], in_=pl_sb)

            nc.sync.dma_start(out=zc, in_=z_sb)

    nc.compile()
    return nc


def kernel(x, W, b):
    x = np.ascontiguousarray(x, dtype=np.float32)
    W = np.ascontiguousarray(W, dtype=np.float32)
    b = np.ascontiguousarray(b, dtype=np.float32)

    if "nc" not in _CACHE:
        _CACHE["nc"] = _build()
    nc = _CACHE["nc"]

    x16 = x.astype(np.float16)
    wt_feed = W.T.astype(np.float16).reshape(DSUB, 128, P).transpose(1, 0, 2).copy()
    in_maps = []
    for c in range(N_CORES):
        xs = x16[:, c * TSH:(c + 1) * TSH, :]            # [B, TSH, D]
        # xn_feed[b, tp, tb, d] = xs[b, tb*128+tp, d]
        xn_feed = np.ascontiguousarray(
            xs.reshape(B, TBLK, 128, D).transpose(0, 2, 1, 3))
        # xt_feed[b, dp, ds, t] = xs[b, t, ds*128+dp]   (t < TFEED only)
        xt_feed = np.ascontiguousarray(
            xs[:, :TFEED].reshape(B, TFEED, DSUB, 128).transpose(0, 3, 2, 1))
        in_maps.append({
            "xn": xn_feed,
            "xt": xt_feed,
            "wt": wt_feed,
            "bb": b[:, None].copy(),
            "idn": np.eye(128, dtype=np.float16),
        })

    _CACHE["in_maps"] = in_maps
    res = run_bass_kernel_spmd(nc, in_maps, core_ids=list(range(N_CORES)))

    att_raw = np.empty((B, P, T), dtype=np.float32)
    Z = np.zeros((B, P), dtype=np.float32)
    pooled = np.zeros((B, P, D), dtype=np.float32)
    for c in range(N_CORES):
        out = res.results[c]
        att_raw[:, :, c * TSH:(c + 1) * TSH] = np.asarray(
            out["att_raw"], dtype=np.float32)
        Z += np.asarray(out["zc"], dtype=np.float32).T
        pooled += out["pooled_c"]

    att = att_raw / Z[:, :, None]
    pooled = pooled / Z[:, :, None]
    return pooled.astype(np.float32), att.astype(np.float32)


if __name__ == "__main__":
    rng = np.random.default_rng(0)
    x = rng.standard_normal((B, T, D), dtype=np.float32)
    W = (rng.uniform(-1, 1, (P, D)) / 32).astype(np.float32)
    b = (rng.uniform(-1, 1, P) / 32).astype(np.float32)
    pooled, att = kernel(x, W, b)
    print(pooled.shape, att.shape, att.sum(-1)[:2, :2])


# revision 12
# speedup vs baseline: 1.0427x; 1.0427x over previous
"""MultiProbeAttentionPooler Trainium2 kernel.

  logits[b,t,p] = x[b,t,:] @ W[p,:] + b[p]
  att = softmax(logits, axis=t)          -> returned as [B, P, T]
  pooled[b,p,d] = sum_t att[b,t,p] x[b,t,d]

Sharding: T is split across the 8 cores (512 tokens each); every core
handles all 16 batches.  Softmax normalization is deferred to the host:
each core returns unnormalized exp(logits) tiles, per-probe partial sums
Z, and unnormalized partial pooled sums; the host reduces/divides.

The TensorE contracts over the partition dim, so the logits matmul needs
x with D on partitions while the pooling matmul needs T on partitions.
The kernel is HBM-bound, so input bytes are minimized: x is fed in fp16
(PE: 1 cycle/row vs 4 for fp32), once naturally and once transposed -
but only 3/4 of the transposed copy comes from HBM; the last quarter is
rebuilt on-chip with PE transpose ops from the natural tiles, trading
idle PE cycles for DMA bytes.  Logits accumulate in fp32 PSUM; exp runs
in fp32 on ScalarE.  fp16's 10-bit mantissa keeps overall error ~5e-4.

Per core, per (batch, T-shard) unit of work:
  - DMA x_nat [512, 1024] (1 MB) and xT [1024, 384] (0.75 MB), fp16,
    one HWDGE ring each (SP / ACT).
  - PE transposes rebuild xT tokens 384:512 from x_nat (fp16 PSUM),
    VectorE copies them into the xT tile.
  - mm1: logitsT[p, t] += WT[dchunk].T @ xT[dchunk]  (8 chunks, fp32 PSUM)
  - ScalarE Exp with per-partition bias: E (fp16) plus its row-sums Z
    (accum_out, fp32) in one pass; E tiles stream out as the att output.
  - E chunks transposed (tiny fp16 PE ops) to [t, p] stationary layout.
  - mm2: pooled_raw[p, d] += eT[tblk].T @ x_nat[tblk]  (fp32 PSUM).
"""

import sys

import numpy as np

for _p in ("/opt/trn_rl_repo", "/root/.axon_site/_ro/trn_rl_repo"):
    if _p not in sys.path:
        sys.path.append(_p)

import concourse.bacc as bacc
import concourse.tile as tile
from concourse import mybir
from concourse.bass_utils import run_bass_kernel_spmd

dt = mybir.dt
AF = mybir.ActivationFunctionType

B, T, D, P = 16, 4096, 1024, 16
N_CORES = 8
TSH = T // N_CORES          # tokens per core (512)
TBLK = TSH // 128           # 128-token blocks per unit (4)
DSUB = D // 128             # 128-wide D chunks (8)
TFEED = 3 * TSH // 4        # transposed tokens fed from host (384)

_CACHE = {}


def _build():
    nc = bacc.Bacc("TRN2", target_bir_lowering=False, debug=False,
                   num_devices=N_CORES)

    # pre-tiled on host: each SBUF partition reads one contiguous run
    xn = nc.dram_tensor("xn", [B, 128, TBLK, D], dt.float16, kind="ExternalInput").ap()
    xt = nc.dram_tensor("xt", [B, 128, DSUB, TFEED], dt.float16, kind="ExternalInput").ap()
    wt = nc.dram_tensor("wt", [128, DSUB, P], dt.float16, kind="ExternalInput").ap()
    bb = nc.dram_tensor("bb", [P, 1], dt.float32, kind="ExternalInput").ap()
    idn = nc.dram_tensor("idn", [128, 128], dt.float16, kind="ExternalInput").ap()

    att_raw = nc.dram_tensor("att_raw", [B, P, TSH], dt.float16, kind="ExternalOutput").ap()
    zc = nc.dram_tensor("zc", [P, B], dt.float32, kind="ExternalOutput").ap()
    pooled_c = nc.dram_tensor("pooled_c", [B, P, D], dt.float32, kind="ExternalOutput").ap()

    with tile.TileContext(nc) as tc:
        with (
            tc.tile_pool(name="consts", bufs=1) as consts,
            tc.tile_pool(name="xn_p", bufs=4) as xn_p,
            tc.tile_pool(name="xt_p", bufs=4) as xt_p,
            tc.tile_pool(name="e_p", bufs=3) as e_p,
            tc.tile_pool(name="et_p", bufs=2) as et_p,
            tc.tile_pool(name="pl_p", bufs=2) as pl_p,
            tc.tile_pool(name="ps_tr", bufs=2, space="PSUM") as ps_tr,
            tc.tile_pool(name="ps_lg", bufs=2, space="PSUM") as ps_lg,
            tc.tile_pool(name="ps_et", bufs=2, space="PSUM") as ps_et,
            tc.tile_pool(name="ps_pl", bufs=1, space="PSUM") as ps_pl,
        ):
            wt_sb = consts.tile([128, DSUB, P], dt.float16)
            b_sb = consts.tile([P, 1], dt.float32)
            id_sb = consts.tile([128, 128], dt.float16)
            z_sb = consts.tile([P, B], dt.float32)
            nc.sync.dma_start(out=wt_sb, in_=wt)
            nc.sync.dma_start(out=b_sb, in_=bb)
            nc.sync.dma_start(out=id_sb, in_=idn)

            for b in range(B):
                # ---- load x slice both ways; separate HWDGE rings ----
                xn_tile = xn_p.tile([128, TBLK, D], dt.float16)
                nc.sync.dma_start(out=xn_tile, in_=xn[b])
                xt_tile = xt_p.tile([128, DSUB, TSH], dt.float16)
                nc.scalar.dma_start(out=xt_tile[:, :, :TFEED], in_=xt[b])

                # ---- rebuild xT tokens TFEED:TSH on-chip (t-block 3) ----
                for dg in range(2):          # groups of 4 D-chunks
                    pt = ps_tr.tile([128, 4, 128], dt.float16)
                    for i in range(4):
                        ds = dg * 4 + i
                        nc.tensor.transpose(
                            pt[:, i],
                            xn_tile[:, TBLK - 1, ds * 128:(ds + 1) * 128],
                            id_sb,
                        )
                    nc.vector.tensor_copy(
                        xt_tile[:, dg * 4:(dg + 1) * 4, TFEED:TSH], pt)

                # ---- mm1: logitsT [P, TSH] ----
                ps_l = ps_lg.tile([P, TSH], dt.float32)
                for ds in range(DSUB):
                    nc.tensor.matmul(ps_l, wt_sb[:, ds], xt_tile[:, ds],
                                     start=(ds == 0), stop=(ds == DSUB - 1))

                # ---- exp(logits + b) -> E (fp16), Z partial via accum_out ----
                e_sb = e_p.tile([P, TSH], dt.float16)
                nc.scalar.activation(e_sb, ps_l, AF.Exp, bias=b_sb, scale=1.0,
                                     accum_out=z_sb[:, b:b + 1])
                nc.gpsimd.dma_start(out=att_raw[b], in_=e_sb)

                # ---- E chunks back to [t, p] (fp16) for mm2 stationary ----
                et_sb = et_p.tile([128, TBLK, P], dt.float16)
                for tb in range(TBLK):
                    pe = ps_et.tile([128, P], dt.float16)
                    nc.tensor.transpose(pe, e_sb[:, tb * 128:(tb + 1) * 128],
                                        id_sb[:P, :P])
                    nc.vector.tensor_copy(et_sb[:, tb], pe)

                # ---- mm2: pooled_raw [P, D] ----
                ps_p = ps_pl.tile([P, D], dt.float32)
                for tb in range(TBLK):
                    for dh in range(2):
                        nc.tensor.matmul(
                            ps_p[:, dh * 512:(dh + 1) * 512],
                            et_sb[:, tb],
                            xn_tile[:, tb, dh * 512:(dh + 1) * 512],
                            start=(tb == 0), stop=(tb == TBLK - 1),
                        )

                pl_sb = pl_p.tile([P, D], dt.float32)
                nc.vector.tensor_copy(pl_sb, ps_p)
                nc.gpsimd.dma_start(out=pooled_c[b], in_=pl_sb)

            nc.gpsimd.dma_start(out=zc, in_=z_sb)

    nc.compile()
    return nc


def kernel(x, W, b):
    x = np.ascontiguousarray(x, dtype=np.float32)
    W = np.ascontiguousarray(W, dtype=np.float32)
    b = np.ascontiguousarray(b, dtype=np.float32)

    if "nc" not in _CACHE:
        _CACHE["nc"] = _build()
    nc = _CACHE["nc"]

    x16 = x.astype(np.float16)
    wt_feed = W.T.astype(np.float16).reshape(DSUB, 128, P).transpose(1, 0, 2).copy()
    in_maps = []
    for c in range(N_CORES):
        xs = x16[:, c * TSH:(c + 1) * TSH, :]            # [B, TSH, D]
        # xn_feed[b, tp, tb, d] = xs[b, tb*128+tp, d]
        xn_feed = np.ascontiguousarray(
            xs.reshape(B, TBLK, 128, D).transpose(0, 2, 1, 3))
        # xt_feed[b, dp, ds, t] = xs[b, t, ds*128+dp]   (t < TFEED only)
        xt_feed = np.ascontiguousarray(
            xs[:, :TFEED].reshape(B, TFEED, DSUB, 128).transpose(0, 3, 2, 1))
        in_maps.append({
            "xn": xn_feed,
            "xt": xt_feed,
            "wt": wt_feed,
            "bb": b[:, None].copy(),
            "idn": np.eye(128, dtype=np.float16),
        })

    _CACHE["in_maps"] = in_maps
    res = run_bass_kernel_spmd(nc, in_maps, core_ids=list(range(N_CORES)))

    att_raw = np.empty((B, P, T), dtype=np.float32)
    Z = np.zeros((B, P), dtype=np.float32)
    pooled = np.zeros((B, P, D), dtype=np.float32)
    for c in range(N_CORES):
        out = res.results[c]
        att_raw[:, :, c * TSH:(c + 1) * TSH] = np.asarray(
            out["att_raw"], dtype=np.float32)
        Z += np.asarray(out["zc"], dtype=np.float32).T
        pooled += out["pooled_c"]

    att = att_raw / Z[:, :, None]
    pooled = pooled / Z[:, :, None]
    return pooled.astype(np.float32), att.astype(np.float32)


if __name__ == "__main__":
    rng = np.random.default_rng(0)
    x = rng.standard_normal((B, T, D), dtype=np.float32)
    W = (rng.uniform(-1, 1, (P, D)) / 32).astype(np.float32)
    b = (rng.uniform(-1, 1, P) / 32).astype(np.float32)
    pooled, att = kernel(x, W, b)
    print(pooled.shape, att.shape, att.sum(-1)[:2, :2])


# revision 22
# speedup vs baseline: 1.1218x; 1.0759x over previous
"""MultiProbeAttentionPooler Trainium2 kernel.

  logits[b,t,p] = x[b,t,:] @ W[p,:] + b[p]
  att = softmax(logits, axis=t)          -> returned as [B, P, T]
  pooled[b,p,d] = sum_t att[b,t,p] x[b,t,d]

Sharding: T is split across the 8 cores (512 tokens each); every core
handles all 16 batches.  Softmax normalization is deferred to the host:
each core returns unnormalized exp(logits) tiles, per-probe partial sums
Z, and unnormalized partial pooled sums; the host reduces/divides.

The TensorE contracts over the partition dim, so the logits matmul needs
x with D on partitions while the pooling matmul needs T on partitions.
The kernel is HBM-bound, so input bytes are minimized: x is fed in fp16
(PE: 1 cycle/row vs 4 for fp32), once naturally and once transposed -
but only 3/4 of the transposed copy comes from HBM; the last quarter is
rebuilt on-chip with PE transpose ops from the natural tiles, trading
idle PE cycles for DMA bytes.  Logits accumulate in fp32 PSUM; exp runs
in fp32 on ScalarE.  fp16's 10-bit mantissa keeps overall error ~5e-4.

Per core, per (batch, T-shard) unit of work:
  - DMA x_nat [512, 1024] (1 MB) and xT [1024, 384] (0.75 MB), fp16,
    one HWDGE ring each (SP / ACT).
  - PE transposes rebuild xT tokens 384:512 from x_nat (fp16 PSUM),
    VectorE copies them into the xT tile.
  - mm1: logitsT[p, t] += WT[dchunk].T @ xT[dchunk]  (8 chunks, fp32 PSUM)
  - ScalarE Exp with per-partition bias: E (fp16) plus its row-sums Z
    (accum_out, fp32) in one pass; E tiles stream out as the att output.
  - E chunks transposed (tiny fp16 PE ops) to [t, p] stationary layout.
  - mm2: pooled_raw[p, d] += eT[tblk].T @ x_nat[tblk]  (fp32 PSUM).
"""

import sys

import numpy as np

for _p in ("/opt/trn_rl_repo", "/root/.axon_site/_ro/trn_rl_repo"):
    if _p not in sys.path:
        sys.path.append(_p)

import concourse.bacc as bacc
import concourse.tile as tile
from concourse import mybir
from concourse.bass_utils import run_bass_kernel_spmd

dt = mybir.dt
AF = mybir.ActivationFunctionType

B, T, D, P = 16, 4096, 1024, 16
N_CORES = 8
TSH = T // N_CORES          # tokens per core (512)
TBLK = TSH // 128           # 128-token blocks per unit (4)
DSUB = D // 128             # 128-wide D chunks (8)
TFEED = 3 * TSH // 4        # transposed tokens fed from host (384)

_CACHE = {}


def _build():
    nc = bacc.Bacc("TRN2", target_bir_lowering=False, debug=False,
                   num_devices=N_CORES)

    # pre-tiled on host: each SBUF partition reads one contiguous run
    xn = nc.dram_tensor("xn", [B, 128, TBLK, D], dt.float16, kind="ExternalInput").ap()
    xt = nc.dram_tensor("xt", [B, 128, DSUB, TFEED], dt.float16, kind="ExternalInput").ap()
    wt = nc.dram_tensor("wt", [128, DSUB, P], dt.float16, kind="ExternalInput").ap()
    bb = nc.dram_tensor("bb", [P, 1], dt.float32, kind="ExternalInput").ap()
    idn = nc.dram_tensor("idn", [128, 128], dt.float16, kind="ExternalInput").ap()

    att_raw = nc.dram_tensor("att_raw", [B, P, TSH], dt.float16, kind="ExternalOutput").ap()
    pooled_c = nc.dram_tensor("pooled_c", [B, P, D], dt.float32, kind="ExternalOutput").ap()

    with tile.TileContext(nc) as tc:
        with (
            tc.tile_pool(name="consts", bufs=1) as consts,
            tc.tile_pool(name="xn_p", bufs=4) as xn_p,
            tc.tile_pool(name="xt_p", bufs=4) as xt_p,
            tc.tile_pool(name="xtb_p", bufs=4) as xtb_p,
            tc.tile_pool(name="e_p", bufs=3) as e_p,
            tc.tile_pool(name="et_p", bufs=2) as et_p,
            tc.tile_pool(name="pl_p", bufs=2) as pl_p,
            tc.tile_pool(name="ps_tr", bufs=1, space="PSUM") as ps_tr,
            tc.tile_pool(name="ps_lg", bufs=2, space="PSUM") as ps_lg,
            tc.tile_pool(name="ps_lb", bufs=1, space="PSUM") as ps_lb,
            tc.tile_pool(name="ps_et", bufs=2, space="PSUM") as ps_et,
            tc.tile_pool(name="ps_pl", bufs=1, space="PSUM") as ps_pl,
        ):
            wt_sb = consts.tile([128, DSUB, P], dt.float16)
            b_sb = consts.tile([P, 1], dt.float32)
            id_sb = consts.tile([128, 128], dt.float16)
            nc.sync.dma_start(out=wt_sb, in_=wt)
            nc.sync.dma_start(out=b_sb, in_=bb)
            nc.sync.dma_start(out=id_sb, in_=idn)

            for b in range(B):
                # ---- load x slice both ways; alternate HWDGE rings ----
                ring_a = nc.sync if b % 2 == 0 else nc.scalar
                ring_b = nc.scalar if b % 2 == 0 else nc.sync
                xn_tile = xn_p.tile([128, TBLK, D], dt.float16)
                ring_a.dma_start(out=xn_tile, in_=xn[b])
                xt_tile = xt_p.tile([128, DSUB, TFEED], dt.float16)
                ring_b.dma_start(out=xt_tile, in_=xt[b])

                # ---- rebuild xT tokens TFEED:TSH on-chip (t-block 3) ----
                xtb_tile = xtb_p.tile([128, DSUB, TSH - TFEED], dt.float16)
                for dg in range(2):          # groups of 4 D-chunks
                    pt = ps_tr.tile([128, 4, 128], dt.float16)
                    for i in range(4):
                        ds = dg * 4 + i
                        nc.tensor.transpose(
                            pt[:, i],
                            xn_tile[:, TBLK - 1, ds * 128:(ds + 1) * 128],
                            id_sb,
                        )
                    nc.vector.tensor_copy(xtb_tile[:, dg * 4:(dg + 1) * 4], pt)

                # ---- mm1: logitsT [P, TSH], two PSUM banks (A: fed, B: rebuilt) ----
                ps_l = ps_lg.tile([P, TFEED], dt.float32)
                for ds in range(DSUB):
                    nc.tensor.matmul(ps_l, wt_sb[:, ds], xt_tile[:, ds],
                                     start=(ds == 0), stop=(ds == DSUB - 1))
                ps_b = ps_lb.tile([P, TSH - TFEED], dt.float32)
                for ds in range(DSUB):
                    nc.tensor.matmul(ps_b, wt_sb[:, ds], xtb_tile[:, ds],
                                     start=(ds == 0), stop=(ds == DSUB - 1))

                # ---- exp(logits + b) -> E (fp16); Z summed on host ----
                e_sb = e_p.tile([P, TSH], dt.float16)
                nc.scalar.activation(e_sb[:, :TFEED], ps_l, AF.Exp,
                                     bias=b_sb, scale=1.0)
                nc.scalar.activation(e_sb[:, TFEED:], ps_b, AF.Exp,
                                     bias=b_sb, scale=1.0)
                nc.gpsimd.dma_start(out=att_raw[b], in_=e_sb)

                # ---- E chunks back to [t, p] (fp16) for mm2 stationary ----
                et_sb = et_p.tile([128, TBLK, P], dt.float16)
                for tb in range(TBLK):
                    pe = ps_et.tile([128, P], dt.float16)
                    nc.tensor.transpose(pe, e_sb[:, tb * 128:(tb + 1) * 128],
                                        id_sb[:P, :P])
                    nc.vector.tensor_copy(et_sb[:, tb], pe)

                # ---- mm2: pooled_raw [P, D] ----
                ps_p = ps_pl.tile([P, D], dt.float32)
                for tb in range(TBLK):
                    for dh in range(2):
                        nc.tensor.matmul(
                            ps_p[:, dh * 512:(dh + 1) * 512],
                            et_sb[:, tb],
                            xn_tile[:, tb, dh * 512:(dh + 1) * 512],
                            start=(tb == 0), stop=(tb == TBLK - 1),
                        )

                pl_sb = pl_p.tile([P, D], dt.float32)
                nc.vector.tensor_copy(pl_sb, ps_p)
                nc.gpsimd.dma_start(out=pooled_c[b], in_=pl_sb)

    nc.compile()
    return nc


def kernel(x, W, b):
    x = np.ascontiguousarray(x, dtype=np.float32)
    W = np.ascontiguousarray(W, dtype=np.float32)
    b = np.ascontiguousarray(b, dtype=np.float32)

    if "nc" not in _CACHE:
        _CACHE["nc"] = _build()
    nc = _CACHE["nc"]

    x16 = x.astype(np.float16)
    wt_feed = W.T.astype(np.float16).reshape(DSUB, 128, P).transpose(1, 0, 2).copy()
    in_maps = []
    for c in range(N_CORES):
        xs = x16[:, c * TSH:(c + 1) * TSH, :]            # [B, TSH, D]
        # xn_feed[b, tp, tb, d] = xs[b, tb*128+tp, d]
        xn_feed = np.ascontiguousarray(
            xs.reshape(B, TBLK, 128, D).transpose(0, 2, 1, 3))
        # xt_feed[b, dp, ds, t] = xs[b, t, ds*128+dp]   (t < TFEED only)
        xt_feed = np.ascontiguousarray(
            xs[:, :TFEED].reshape(B, TFEED, DSUB, 128).transpose(0, 3, 2, 1))
        in_maps.append({
            "xn": xn_feed,
            "xt": xt_feed,
            "wt": wt_feed,
            "bb": b[:, None].copy(),
            "idn": np.eye(128, dtype=np.float16),
        })

    _CACHE["in_maps"] = in_maps
    res = run_bass_kernel_spmd(nc, in_maps, core_ids=list(range(N_CORES)))

    att_raw = np.empty((B, P, T), dtype=np.float32)
    pooled = np.zeros((B, P, D), dtype=np.float32)
    for c in range(N_CORES):
        out = res.results[c]
        att_raw[:, :, c * TSH:(c + 1) * TSH] = np.asarray(
            out["att_raw"], dtype=np.float32)
        pooled += out["pooled_c"]
    Z = att_raw.sum(axis=-1)

    att = att_raw / Z[:, :, None]
    pooled = pooled / Z[:, :, None]
    return pooled.astype(np.float32), att.astype(np.float32)


if __name__ == "__main__":
    rng = np.random.default_rng(0)
    x = rng.standard_normal((B, T, D), dtype=np.float32)
    W = (rng.uniform(-1, 1, (P, D)) / 32).astype(np.float32)
    b = (rng.uniform(-1, 1, P) / 32).astype(np.float32)
    pooled, att = kernel(x, W, b)
    print(pooled.shape, att.shape, att.sum(-1)[:2, :2])
